# revision 28
# baseline (speedup 1.0000x reference)
"""Multi-head attention Trainium2 kernel (Bass/Tile, SPMD over 8 cores).

fp16 compute variant: matmul operands in fp16 (2-byte stream rate = 2x fp32),
fp32 PSUM accumulation, fp32 normalization. Rel err vs fp32 reference ~1e-3.

Sharding: data parallel over batch. Core i computes batches [2i, 2i+2).

Structure per core:
  - Host pre-transposes x -> xT [d, s] and weights -> [d, h*e]; contiguous DMAs.
  - v projections for BOTH batches run first (covers weight-DMA latency).
  - qT/kT per head-pair: lhsT=W chunk, rhs=xT chunk, accumulate 8 d-chunks.
  - Scores transposed ST[t,s]; exp on ScalarE with bias=-5 (fp16 headroom);
    the two heads of a pair sit at base partitions 0/64 so their K=64
    matmuls row-pack concurrently on the PE.
  - PV with V_aug stationary (ones column -> denominators ride along):
    out^T[e,s] in PSUM; scaled 1/16 copy to fp16 SBUF; PE-transpose back to
    [s,e]; per-partition reciprocal multiply (the 1/16 cancels exactly).
"""

import numpy as np

import concourse.bass as bass
import concourse.mybir as mybir
import concourse.tile as tile
from concourse.bass_utils import run_bass_kernel_spmd
from concourse.masks import make_identity

B, S, D, H, DH = 16, 512, 1024, 16, 64
N_CORES = 8
B_LOC = B // N_CORES  # 2 batches per core
C = D // 128  # 8 contraction chunks over d
TC = S // 128  # 4 chunks over s/t
F32 = mybir.dt.float32
FP16 = mybir.dt.float16
SCALE = 1.0 / np.sqrt(np.float32(D))
EXP_BIAS = -5.0  # exp(logit-5): keeps P in fp16 range; cancels in normalize
OSCALE = 1.0 / 16.0  # pre-scale before fp16 transpose; cancels in normalize


def legalize_waits(nc, cap=1):
    """This walrus build supports at most `cap` sync-wait commands per
    instruction; hoist excess waits onto preceding same-engine NoOps."""
    n_split = 0
    for f in nc.m.functions:
        for blk in f.blocks:
            new_insts = []
            for inst in blk.instructions:
                si = getattr(inst, "sync_info", None)
                waits = list(si.on_wait) if si is not None and si.on_wait else []
                if len(waits) > cap:
                    keep, rest = waits[:cap], waits[cap:]
                    while rest:
                        chunk, rest = rest[:cap], rest[cap:]
                        nop = mybir.InstNoOp(
                            name=f"I-waitsplit-{nc.next_id()}", ins=[], outs=[]
                        )
                        nop.engine = inst.engine
                        nop.sync_info = mybir.SyncInfo(on_wait=chunk, on_update=[])
                        nc.register_instruction(nop, overwrite=True)
                        new_insts.append(nop)
                        n_split += 1
                    si.on_wait = keep
                new_insts.append(inst)
            blk.instructions[:] = new_insts
    return n_split


def build_program():
    nc = bass.Bass()
    xt_d = nc.declare_dram_parameter("xt", [B_LOC, C, 128, S], FP16, isOutput=False)
    wq_d = nc.declare_dram_parameter("wq", [C, 128, D], FP16, isOutput=False)
    wk_d = nc.declare_dram_parameter("wk", [C, 128, D], FP16, isOutput=False)
    wv_d = nc.declare_dram_parameter("wv", [C, 128, D], FP16, isOutput=False)
    out_d = nc.declare_dram_parameter("out", [B_LOC, S, D], F32, isOutput=True)

    with tile.TileContext(nc) as tc:
        with (
            tc.tile_pool(name="wpool", bufs=1) as wpool,
            tc.tile_pool(name="xpool", bufs=1) as xpool,
            tc.tile_pool(name="vpool", bufs=4) as vpool,
            tc.tile_pool(name="qkpool", bufs=4) as qkpool,
            tc.tile_pool(name="ppool", bufs=10) as ppool,
            tc.tile_pool(name="opool", bufs=4) as opool,
            tc.tile_pool(name="rpool", bufs=8) as rpool,
            tc.tile_pool(name="ovpool", bufs=3) as ovpool,
            tc.tile_pool(name="psmm", bufs=1, space="PSUM") as psmm,
            tc.tile_pool(name="stp", bufs=2, space="PSUM") as stp,
            tc.tile_pool(name="psout", bufs=2, space="PSUM") as psout,
            tc.tile_pool(name="pstr", bufs=1, space="PSUM") as pstr,
        ):
            ident = ovpool.tile([128, 128], FP16, tag="ident", bufs=1)
            make_identity(nc, ident)
            exp_bias = ovpool.tile([128, 1], F32, tag="expbias", bufs=1)
            nc.vector.memset(exp_bias, EXP_BIAS)

            # ---- inputs: xT both batches first, then wv, then wq/wk ----
            xts = [
                xpool.tile([128, C, S], FP16, tag=f"xt{b}", name=f"xt{b}")
                for b in range(B_LOC)
            ]
            wq_sb = wpool.tile([128, C, D], FP16, tag="wq")
            wk_sb = wpool.tile([128, C, D], FP16, tag="wk")
            wv_sb = wpool.tile([128, C, D], FP16, tag="wv")
            for c in range(C):
                nc.sync.dma_start(out=xts[0][:, c, :], in_=xt_d[0, c])
                nc.sync.dma_start(out=wv_sb[:, c, :], in_=wv_d[c])
            for c in range(C):
                nc.sync.dma_start(out=wq_sb[:, c, :], in_=wq_d[c])
                nc.sync.dma_start(out=wk_sb[:, c, :], in_=wk_d[c])
            for c in range(C):
                nc.sync.dma_start(out=xts[1][:, c, :], in_=xt_d[1, c])

            # ---- v projections for BOTH batches up front ----
            # V_aug layout [128(t), h, 64(e) + ones + pad]
            vaugs = {}
            for b in range(B_LOC):
                vaugs[b] = [
                    vpool.tile(
                        [128, H, DH + 2], FP16, tag=f"vaug{b}", name=f"vaug{b}_{t}"
                    )
                    for t in range(TC)
                ]
                for t in range(TC):
                    nc.vector.memset(vaugs[b][t][:, :, DH : DH + 2], 1.0)
            for b in range(B_LOC):
                for t in range(TC):
                    for half in range(2):
                        ps = psmm.tile([128, 512], F32, tag="mm")
                        for c in range(C):
                            nc.tensor.matmul(
                                ps,
                                lhsT=xts[b][:, c, t * 128 : (t + 1) * 128],
                                rhs=wv_sb[:, c, half * 512 : (half + 1) * 512],
                                start=(c == 0),
                                stop=(c == C - 1),
                            )
                        nc.vector.tensor_copy(
                            vaugs[b][t][:, half * 8 : (half + 1) * 8, 0:DH],
                            ps.rearrange("p (h e) -> p h e", h=8),
                        )

            # ---- per-batch, per-head-pair attention ----
            for b in range(B_LOC):
                xt_sb = xts[b]
                vaug = vaugs[b]
                osb = [
                    opool.tile([128, D], F32, tag="osb", name=f"osb{b}_{sc}")
                    for sc in range(TC)
                ]
                for pair in range(H // 2):
                    qt = qkpool.tile([128, S], FP16, tag="qt")
                    kt = qkpool.tile([128, S], FP16, tag="kt")
                    for w_sb, dst in ((wq_sb, qt), (wk_sb, kt)):
                        ps = psmm.tile([128, 512], F32, tag="mm")
                        for c in range(C):
                            nc.tensor.matmul(
                                ps,
                                lhsT=w_sb[:, c, pair * 128 : (pair + 1) * 128],
                                rhs=xt_sb[:, c, :],
                                start=(c == 0),
                                stop=(c == C - 1),
                            )
                        nc.vector.tensor_copy(dst, ps)

                    # ST matmuls (t, half)-interleaved so the two K=64
                    # halves (base partitions 0/64) row-pack on the PE; the
                    # two halves land in one 2-bank psum tile so a single
                    # [128,1024] exp covers both (fewer ACT dispatches)
                    p_tiles = {}
                    for t in range(TC):
                        ps = stp.tile([128, 2, 512], F32, tag="st")
                        for half in range(2):
                            lo, hi = 64 * half, 64 * (half + 1)
                            nc.tensor.matmul(
                                ps[:, half, :],
                                lhsT=kt[lo:hi, t * 128 : (t + 1) * 128],
                                rhs=qt[lo:hi, :],
                                start=True,
                                stop=True,
                            )
                        pt = ppool.tile([128, 2, 512], FP16, tag="p")
                        nc.scalar.activation(
                            pt.rearrange("p a b -> p (a b)"),
                            ps.rearrange("p a b -> p (a b)"),
                            mybir.ActivationFunctionType.Exp,
                            scale=float(SCALE),
                            bias=exp_bias[:, :],
                        )
                        for half in range(2):
                            p_tiles[(half, t)] = pt[:, half, :]

                    for half in range(2):
                        h = pair * 2 + half
                        oaug = psout.tile([DH + 2, 512], F32, tag="o")
                        for t in range(TC):
                            nc.tensor.matmul(
                                oaug,
                                lhsT=vaug[t][:, h, :],
                                rhs=p_tiles[(half, t)],
                                start=(t == 0),
                                stop=(t == TC - 1),
                            )
                        # scaled fp16 copy: transposes stream 2x faster; the
                        # 1/16 cancels in num/denom
                        oaug_sb = ovpool.tile([DH + 2, 512], FP16, tag="oaug")
                        nc.scalar.mul(oaug_sb, oaug, OSCALE)
                        for sc in range(TC):
                            ot = pstr.tile([128, DH + 2], FP16, tag="ot")
                            nc.tensor.transpose(
                                ot,
                                oaug_sb[:, sc * 128 : (sc + 1) * 128],
                                ident[: DH + 2, : DH + 2],
                            )
                            recip = rpool.tile([128, 1], F32, tag="r")
                            nc.vector.reciprocal(recip, ot[:, DH : DH + 1])
                            nc.vector.tensor_scalar_mul(
                                osb[sc][:, h * DH : (h + 1) * DH], ot[:, 0:DH], recip
                            )

                    # stream this pair's output columns out now; keeps the
                    # final DMA tail short
                    for sc in range(TC):
                        nc.sync.dma_start(
                            out=out_d[b, sc * 128 : (sc + 1) * 128, pair * 128 : (pair + 1) * 128],
                            in_=osb[sc][:, pair * 128 : (pair + 1) * 128],
                        )


    legalize_waits(nc)
    return nc


def _prep_inputs(x, Wq, Wk, Wv):
    x = np.ascontiguousarray(np.asarray(x, dtype=np.float32))
    # x [B, S, D] -> per-core xT [B_LOC, C, 128, S]
    xt = x.reshape(N_CORES, B_LOC, S, D).transpose(0, 1, 3, 2)
    xt = np.ascontiguousarray(xt).reshape(N_CORES, B_LOC, C, 128, S).astype(np.float16)
    wp = []
    for W in (Wq, Wk, Wv):
        W = np.asarray(W, dtype=np.float32)
        # [H, D, DH] -> [D, H*DH] (d-major) -> [C, 128, H*DH]
        wp.append(
            np.ascontiguousarray(W.transpose(1, 0, 2))
            .reshape(C, 128, H * DH)
            .astype(np.float16)
        )
    return xt, wp[0], wp[1], wp[2]


_PROGRAM = None


def _get_program():
    global _PROGRAM
    if _PROGRAM is None:
        _PROGRAM = build_program()
    return _PROGRAM


def run(x, Wq, Wk, Wv, trace=False, nc=None):
    xt, wq_p, wk_p, wv_p = _prep_inputs(x, Wq, Wk, Wv)
    if nc is None:
        nc = _get_program()
    in_maps = [
        {"xt": xt[i], "wq": wq_p, "wk": wk_p, "wv": wv_p} for i in range(N_CORES)
    ]
    res = run_bass_kernel_spmd(nc, in_maps, list(range(N_CORES)), trace=trace)
    out = np.concatenate([res.results[i]["out"] for i in range(N_CORES)], axis=0)
    return out, res


def kernel(x, Wq, Wk, Wv):
    out, _ = run(x, Wq, Wk, Wv, trace=False)
    return out


# revision 29
# speedup vs baseline: 1.2039x; 1.2039x over previous
"""Multi-head attention Trainium2 kernel (Bass/Tile, SPMD over 8 cores).

fp16 compute variant: matmul operands in fp16 (2-byte stream rate = 2x fp32),
fp32 PSUM accumulation, fp32 normalization. Rel err vs fp32 reference ~1e-3.

Sharding: data parallel over batch. Core i computes batches [2i, 2i+2).

Structure per core:
  - Host pre-transposes x -> xT [d, s] and weights -> [d, h*e]; contiguous DMAs.
  - v projections for BOTH batches run first (covers weight-DMA latency).
  - qT/kT per head-pair: lhsT=W chunk, rhs=xT chunk, accumulate 8 d-chunks.
  - Scores transposed ST[t,s]; exp on ScalarE with bias=-5 (fp16 headroom);
    the two heads of a pair sit at base partitions 0/64 so their K=64
    matmuls row-pack concurrently on the PE.
  - PV with V_aug stationary (ones column -> denominators ride along):
    out^T[e,s] in PSUM; scaled 1/16 copy to fp16 SBUF; PE-transpose back to
    [s,e]; per-partition reciprocal multiply (the 1/16 cancels exactly).
"""

import numpy as np

import concourse.bass as bass
import concourse.mybir as mybir
import concourse.tile as tile
from concourse.bass_utils import run_bass_kernel_spmd
from concourse.masks import make_identity

B, S, D, H, DH = 16, 512, 1024, 16, 64
N_CORES = 8
B_LOC = B // N_CORES  # 2 batches per core
C = D // 128  # 8 contraction chunks over d
TC = S // 128  # 4 chunks over s/t
F32 = mybir.dt.float32
FP16 = mybir.dt.float16
SCALE = 1.0 / np.sqrt(np.float32(D))
EXP_BIAS = -5.0  # exp(logit-5): keeps P in fp16 range; cancels in normalize
OSCALE = 1.0 / 16.0  # pre-scale before fp16 transpose; cancels in normalize


def legalize_waits(nc, cap=1):
    """This walrus build supports at most `cap` sync-wait commands per
    instruction; hoist excess waits onto preceding same-engine NoOps."""
    n_split = 0
    for f in nc.m.functions:
        for blk in f.blocks:
            new_insts = []
            for inst in blk.instructions:
                si = getattr(inst, "sync_info", None)
                waits = list(si.on_wait) if si is not None and si.on_wait else []
                if len(waits) > cap:
                    keep, rest = waits[:cap], waits[cap:]
                    while rest:
                        chunk, rest = rest[:cap], rest[cap:]
                        nop = mybir.InstNoOp(
                            name=f"I-waitsplit-{nc.next_id()}", ins=[], outs=[]
                        )
                        nop.engine = inst.engine
                        nop.sync_info = mybir.SyncInfo(on_wait=chunk, on_update=[])
                        nc.register_instruction(nop, overwrite=True)
                        new_insts.append(nop)
                        n_split += 1
                    si.on_wait = keep
                new_insts.append(inst)
            blk.instructions[:] = new_insts
    return n_split


def build_program():
    nc = bass.Bass()
    xt_d = nc.declare_dram_parameter("xt", [B_LOC, C, 128, S], FP16, isOutput=False)
    wq_d = nc.declare_dram_parameter("wq", [C, 128, D], FP16, isOutput=False)
    wk_d = nc.declare_dram_parameter("wk", [C, 128, D], FP16, isOutput=False)
    wv_d = nc.declare_dram_parameter("wv", [C, 128, D], FP16, isOutput=False)
    out_d = nc.declare_dram_parameter("out", [B_LOC, S, D], F32, isOutput=True)

    with tile.TileContext(nc) as tc:
        with (
            tc.tile_pool(name="wpool", bufs=1) as wpool,
            tc.tile_pool(name="xpool", bufs=1) as xpool,
            tc.tile_pool(name="vpool", bufs=4) as vpool,
            tc.tile_pool(name="qkpool", bufs=4) as qkpool,
            tc.tile_pool(name="ppool", bufs=10) as ppool,
            tc.tile_pool(name="opool", bufs=4) as opool,
            tc.tile_pool(name="rpool", bufs=8) as rpool,
            tc.tile_pool(name="ovpool", bufs=3) as ovpool,
            tc.tile_pool(name="psmm", bufs=2, space="PSUM") as psmm,
            tc.tile_pool(name="stp", bufs=2, space="PSUM") as stp,
            tc.tile_pool(name="psout", bufs=1, space="PSUM") as psout,
            tc.tile_pool(name="pstr", bufs=1, space="PSUM") as pstr,
        ):
            ident = ovpool.tile([128, 128], FP16, tag="ident", bufs=1)
            make_identity(nc, ident)
            exp_bias = ovpool.tile([128, 1], F32, tag="expbias", bufs=1)
            nc.vector.memset(exp_bias, EXP_BIAS)

            # ---- inputs: xT both batches first, then wv, then wq/wk ----
            xts = [
                xpool.tile([128, C, S], FP16, tag=f"xt{b}", name=f"xt{b}")
                for b in range(B_LOC)
            ]
            wq_sb = wpool.tile([128, C, D], FP16, tag="wq")
            wk_sb = wpool.tile([128, C, D], FP16, tag="wk")
            wv_sb = wpool.tile([128, C, D], FP16, tag="wv")
            for c in range(C):
                nc.sync.dma_start(out=xts[0][:, c, :], in_=xt_d[0, c])
                nc.sync.dma_start(out=wv_sb[:, c, :], in_=wv_d[c])
            for c in range(C):
                nc.sync.dma_start(out=wq_sb[:, c, :], in_=wq_d[c])
                nc.sync.dma_start(out=wk_sb[:, c, :], in_=wk_d[c])
            for c in range(C):
                nc.sync.dma_start(out=xts[1][:, c, :], in_=xt_d[1, c])

            # ---- v projections for BOTH batches up front ----
            # V_aug layout [128(t), h, 64(e) + ones + pad]
            vaugs = {}
            for b in range(B_LOC):
                vaugs[b] = [
                    vpool.tile(
                        [128, H, DH + 2], FP16, tag=f"vaug{b}", name=f"vaug{b}_{t}"
                    )
                    for t in range(TC)
                ]
                for t in range(TC):
                    nc.vector.memset(vaugs[b][t][:, :, DH : DH + 2], 1.0)
            for b in range(B_LOC):
                for t in range(TC):
                    for half in range(2):
                        ps = psmm.tile([128, 512], F32, tag="mm")
                        for c in range(C):
                            nc.tensor.matmul(
                                ps,
                                lhsT=xts[b][:, c, t * 128 : (t + 1) * 128],
                                rhs=wv_sb[:, c, half * 512 : (half + 1) * 512],
                                start=(c == 0),
                                stop=(c == C - 1),
                            )
                        nc.vector.tensor_copy(
                            vaugs[b][t][:, half * 8 : (half + 1) * 8, 0:DH],
                            ps.rearrange("p (h e) -> p h e", h=8),
                        )

            # ---- per-batch, per-head-pair attention ----
            for b in range(B_LOC):
                xt_sb = xts[b]
                vaug = vaugs[b]
                osb = [
                    opool.tile([128, D], F32, tag="osb", name=f"osb{b}_{sc}")
                    for sc in range(TC)
                ]
                for pair in range(H // 2):
                    qt = qkpool.tile([128, S], FP16, tag="qt")
                    kt = qkpool.tile([128, S], FP16, tag="kt")
                    for w_sb, dst in ((wq_sb, qt), (wk_sb, kt)):
                        ps = psmm.tile([128, 512], F32, tag="mm")
                        for c in range(C):
                            nc.tensor.matmul(
                                ps,
                                lhsT=w_sb[:, c, pair * 128 : (pair + 1) * 128],
                                rhs=xt_sb[:, c, :],
                                start=(c == 0),
                                stop=(c == C - 1),
                            )
                        nc.vector.tensor_copy(dst, ps)

                    # ST matmuls (t, half)-interleaved so the two K=64
                    # halves (base partitions 0/64) row-pack on the PE; the
                    # two halves land in one 2-bank psum tile so a single
                    # [128,1024] exp covers both (fewer ACT dispatches)
                    p_tiles = {}
                    for t in range(TC):
                        ps = stp.tile([128, 2, 512], F32, tag="st")
                        for half in range(2):
                            lo, hi = 64 * half, 64 * (half + 1)
                            nc.tensor.matmul(
                                ps[:, half, :],
                                lhsT=kt[lo:hi, t * 128 : (t + 1) * 128],
                                rhs=qt[lo:hi, :],
                                start=True,
                                stop=True,
                            )
                        pt = ppool.tile([128, 2, 512], FP16, tag="p")
                        nc.scalar.activation(
                            pt.rearrange("p a b -> p (a b)"),
                            ps.rearrange("p a b -> p (a b)"),
                            mybir.ActivationFunctionType.Exp,
                            scale=float(SCALE),
                            bias=exp_bias[:, :],
                        )
                        for half in range(2):
                            p_tiles[(half, t)] = pt[:, half, :]

                    for half in range(2):
                        h = pair * 2 + half
                        oaug = psout.tile([DH + 2, 512], F32, tag="o")
                        for t in range(TC):
                            nc.tensor.matmul(
                                oaug,
                                lhsT=vaug[t][:, h, :],
                                rhs=p_tiles[(half, t)],
                                start=(t == 0),
                                stop=(t == TC - 1),
                            )
                        # scaled fp16 copy: transposes stream 2x faster; the
                        # 1/16 cancels in num/denom
                        oaug_sb = ovpool.tile([DH + 2, 512], FP16, tag="oaug")
                        nc.scalar.mul(oaug_sb, oaug, OSCALE)
                        for sc in range(TC):
                            ot = pstr.tile([128, DH + 2], FP16, tag="ot")
                            nc.tensor.transpose(
                                ot,
                                oaug_sb[:, sc * 128 : (sc + 1) * 128],
                                ident[: DH + 2, : DH + 2],
                            )
                            recip = rpool.tile([128, 1], F32, tag="r")
                            nc.vector.reciprocal(recip, ot[:, DH : DH + 1])
                            nc.vector.tensor_scalar_mul(
                                osb[sc][:, h * DH : (h + 1) * DH], ot[:, 0:DH], recip
                            )

                    # stream this pair's output columns out now; keeps the
                    # final DMA tail short
                    for sc in range(TC):
                        nc.sync.dma_start(
                            out=out_d[b, sc * 128 : (sc + 1) * 128, pair * 128 : (pair + 1) * 128],
                            in_=osb[sc][:, pair * 128 : (pair + 1) * 128],
                        )


    legalize_waits(nc)
    return nc


def _prep_inputs(x, Wq, Wk, Wv):
    x = np.ascontiguousarray(np.asarray(x, dtype=np.float32))
    # x [B, S, D] -> per-core xT [B_LOC, C, 128, S]
    xt = x.reshape(N_CORES, B_LOC, S, D).transpose(0, 1, 3, 2)
    xt = np.ascontiguousarray(xt).reshape(N_CORES, B_LOC, C, 128, S).astype(np.float16)
    wp = []
    for W in (Wq, Wk, Wv):
        W = np.asarray(W, dtype=np.float32)
        # [H, D, DH] -> [D, H*DH] (d-major) -> [C, 128, H*DH]
        wp.append(
            np.ascontiguousarray(W.transpose(1, 0, 2))
            .reshape(C, 128, H * DH)
            .astype(np.float16)
        )
    return xt, wp[0], wp[1], wp[2]


_PROGRAM = None


def _get_program():
    global _PROGRAM
    if _PROGRAM is None:
        _PROGRAM = build_program()
    return _PROGRAM


def run(x, Wq, Wk, Wv, trace=False, nc=None):
    xt, wq_p, wk_p, wv_p = _prep_inputs(x, Wq, Wk, Wv)
    if nc is None:
        nc = _get_program()
    in_maps = [
        {"xt": xt[i], "wq": wq_p, "wk": wk_p, "wv": wv_p} for i in range(N_CORES)
    ]
    res = run_bass_kernel_spmd(nc, in_maps, list(range(N_CORES)), trace=trace)
    out = np.concatenate([res.results[i]["out"] for i in range(N_CORES)], axis=0)
    return out, res


def kernel(x, Wq, Wk, Wv):
    out, _ = run(x, Wq, Wk, Wv, trace=False)
    return out
